# revision 6
# baseline (speedup 1.0000x reference)
"""HXE loss kernel for Trainium2 (8 NeuronCores, batch-sharded).

Math: for a balanced 8-ary tree of depth 4 over C=4096 leaves, the
reference's onehot_num[t, c, j] is the indicator "c lies in the same
contiguous 8**j block as t", and onehot_den[t, c, j] = same at 8**(j+1)
(all-ones at j=3).  Hence with e = exp(logits) (softmax numerators; the
1/Z factors cancel in num/den ratios):

    num[b, j] = S_j(b),  den[b, j] = S_{j+1}(b)
    S_j(b)    = sum of e[b, c] over the 8**j block containing t_b
    S_4(b)    = sum_c e[b, c]

    loss = mean_b sum_j w[t_b, j] * (log S_{j+1} - log S_j)

The device computes the memory-bound part: exp over the full [B, C]
logits and all 8-wide block partial sums.  Each sample's target logit is
also packed (by the host) into an extra 8-wide block padded with -100
(exp -> 0), so S_0 = exp(target logit) falls out of the same exp+reduce
pass.  The host does the target-indexed selection, logs, weighting and
the final mean (the gather / all-reduce step of the sharded execution).

Layout per core (32 samples): partition p = 4*b + k holds quarter k
(1024 classes) of sample b, plus the 8 extra columns; free dim 1032.
"""

import numpy as np

_B, _C = 256, 4096
_NCORES = 8
_BS = _B // _NCORES          # 32 samples per core
_K = 4                       # quarters per sample -> 4*32 = 128 partitions
_M = _C // _K                # 1024 class columns per partition
_W = 8                       # block width reduced on device
_MX = _M + _W                # + extra block carrying the target logit
_NB = _MX // _W              # 129 block sums per partition
_CHUNKS = (256, 256, 256, 264)
_PAD = -100.0                # exp(-100) == 0 in f32

_module_cache = {}


def _build_module():
    # Raw Bass (no TileContext): the Tile kernel-tail Drain aggregates one
    # wait per used semaphore lane and trips walrus's per-instruction sync
    # wait limit, so we hand-roll the (tiny) synchronization instead.
    import concourse.bass as bass
    from concourse import mybir

    nc = bass.Bass("TRN2", target_bir_lowering=False, debug=False)
    x = nc.dram_tensor("x", [128, _MX], mybir.dt.float32, kind="ExternalInput").ap()
    s1 = nc.dram_tensor("s1", [128, _NB], mybir.dt.float32, kind="ExternalOutput").ap()

    nch = len(_CHUNKS)
    offs = []
    col = 0
    for cw in _CHUNKS:
        offs.append((col, cw))
        col += cw

    with (
        nc.sbuf_tensor([128, _MX], mybir.dt.float32) as xt,
        nc.sbuf_tensor([128, _MX], mybir.dt.float32) as et,
        nc.sbuf_tensor([128, _NB], mybir.dt.float32) as s1t,
        nc.semaphore() as dma_sem,
        nc.semaphore() as a_sem,
        nc.semaphore() as v_sem,
        nc.Block() as block,
    ):

        @block.sync
        def _(sync):
            for col, cw in offs:
                sync.dma_start(
                    out=xt[:, col : col + cw], in_=x[:, col : col + cw]
                ).then_inc(dma_sem, 16)
            sync.wait_ge(v_sem, nch)
            sync.dma_start(out=s1, in_=s1t[:, :]).then_inc(dma_sem, 16)
            sync.wait_ge(dma_sem, 16 * (nch + 1))

        @block.scalar
        def _(scalar):
            for i, (col, cw) in enumerate(offs):
                scalar.wait_ge(dma_sem, 16 * (i + 1))
                scalar.activation(
                    out=et[:, col : col + cw],
                    in_=xt[:, col : col + cw],
                    func=mybir.ActivationFunctionType.Exp,
                ).then_inc(a_sem, 1)

        @block.vector
        def _(vector):
            for i, (col, cw) in enumerate(offs):
                vector.wait_ge(a_sem, i + 1)
                vector.reduce_sum(
                    out=s1t[:, col // _W : (col + cw) // _W],
                    in_=et[:, col : col + cw].rearrange("p (n w) -> p n w", w=_W),
                    axis=mybir.AxisListType.X,
                ).then_inc(v_sem, 1)

    return nc


def _get_module():
    if "nc" not in _module_cache:
        _module_cache["nc"] = _build_module()
    return _module_cache["nc"]


def _run_device(logits, t, trace=False, **kwargs):
    """Shard logits over the 8 cores, run the bass kernel, return
    (s1_full [B, C//_W], s0_full [B]) raw-exp block sums, plus results."""
    from concourse import bass_utils

    nc = _get_module()
    logits = np.ascontiguousarray(logits, dtype=np.float32)
    in_maps = []
    for c in range(_NCORES):
        sl = slice(c * _BS, (c + 1) * _BS)
        shard = logits[sl]                              # [32, 4096]
        xbuf = np.full((128, _MX), _PAD, dtype=np.float32)
        xbuf[:, :_M] = shard.reshape(128, _M)
        xbuf[0::_K, _M] = shard[np.arange(_BS), t[sl]]  # target logit
        in_maps.append({"x": xbuf})
    res = bass_utils.run_bass_kernel_spmd(
        nc, in_maps, core_ids=list(range(_NCORES)), trace=trace, **kwargs
    )
    s1 = np.concatenate(
        [r["s1"].reshape(_BS, _K, _NB)[:, :, : _M // _W].reshape(_BS, _C // _W)
         for r in res.results],
        axis=0,
    )
    s0 = np.concatenate(
        [r["s1"].reshape(_BS, _K, _NB)[:, 0, _M // _W] for r in res.results]
    )
    return s1, s0, res


def _finish_host(s1, s0, t, weights):
    """Selection + logs + weighted mean (float64 on host)."""
    b = np.arange(_B)
    s1 = s1.astype(np.float64)                    # [B, 512] 8-block sums
    s64 = s1.reshape(_B, 64, 8).sum(axis=2)       # 64-block sums
    s512 = s64.reshape(_B, 8, 8).sum(axis=2)      # 512-block sums
    z = s512.sum(axis=1)                          # full-row sums

    num = np.stack(
        [s0.astype(np.float64), s1[b, t // 8], s64[b, t // 64], s512[b, t // 512]],
        axis=1,
    )                                             # [B, 4] = S_0..S_3
    den = np.stack([s1[b, t // 8], s64[b, t // 64], s512[b, t // 512], z], axis=1)

    mask = num != 0
    val = np.where(mask, np.log(np.where(mask, den, 1.0) / np.where(mask, num, 1.0)), 0.0)
    w = weights[t].astype(np.float64)             # [B, 4], as the reference gathers
    return (w * val).sum(axis=1).mean()


def kernel(logits, level_wise_target, onehot_num, onehot_den, weights):
    t = np.asarray(level_wise_target)[:, -1].astype(np.int64)
    s1, s0, _ = _run_device(np.asarray(logits), t)
    loss = _finish_host(s1, s0, t, np.asarray(weights))
    return np.asarray(loss, dtype=np.float32)
